# revision 18
# baseline (speedup 1.0000x reference)
"""ChiSquareLoss kernel for Trainium2 (8 NeuronCores, SPMD).

Problem (see reference): for each of B=16384 rows of a [B, 2048] f32 matrix,
build a 10-bin histogram between the row's min and max, then
chi2_row = sum_j (obs_j - e)^2 / (e + eps) with e = B/10, and return
mean(chi2_row).

Per row: bin index of x is #{k in 1..9 : x > b_k}, b_k = mn + (mx-mn)*k/10,
so the histogram follows from the cumulative counts c_k = #{x > b_k}:
obs_j = c_j - c_{j+1}, c_0 = 2048, c_10 = 0.

Engine strategy (each core: 16 tiles of [128 rows, 2048]):
  All DVE reduction-class ops run at 1x (~2.3-2.4us per [128,2048] pass) on
  TRN2 regardless of dtype (accumulate/scan uops exist only for 1x), ACT at
  (N+352)/1.2ns + 280ns accumulator read, so the kernel minimizes the NUMBER
  of full passes over the data using two runtime-registered custom DVE ops:

  MINMAX_SCAN (1 pass = row min AND max): out[k] = parity-select between
    running-min and running-max scans; the tile carries one extra sentinel
    column (-3e38, neutral for max) so col 2047 (odd) holds min of all 2048
    elements and col 2048 (even) holds max. One strided 2-column harvest
    copy recovers [mn, mx].

  SCAN3 (1 pass = 3 counts): out[k] = prefix-sum of
    (x>b_lo) + 256*((x>b_mid) + 256*(x>b_hi)); harvesting the prefix at the
    16 page boundaries (pages of 128) and differencing gives per-page packed
    counts, exact in fp32 because per-page fields (<=128) stay below 256 and
    the row-level prefix stays < 2^24 when the mid/high fields are assigned
    to the small-count (upper) boundaries. Groups: (b1,b7,b8) and (b2,b4,b9)
    on DVE; b3, b5, b6 counted on ACT via the Sign-activation accumulator
    (c = 1024 - S/2).

  Per tile: DVE = minmax_scan + 2 scan3 + 3 tiny harvests (~7.8us),
  ACT = boundary affine ops + 3 sign passes (~7.2us); a one-tile software
  pipeline keeps both engines busy. Epilogue: bulk page-diff + field decode
  (magic-constant rounding) + per-tile page sums -> c_k, obs diffs, one ACT
  Square pass with accumulator -> per-partition partial sums.
  Host: total / (e + eps) / B.
"""

import numpy as np

_B_FULL = 16384
_D = 2048
_N_CORES = 8
_ROWS_PER_CORE = _B_FULL // _N_CORES  # 2048
_P = 128
_TILES = _ROWS_PER_CORE // _P  # 16
_BINS = 10
_E_F32 = np.float32(_B_FULL / _BINS)  # 1638.4f

_MAGIC = float(np.float32(2 ** 23 + 2 ** 22))  # round-to-int magic for fp32
_M = 256.0        # scan3 packing multiplier
_PAGES = 16       # pages of 128 elements
_PGSZ = _D // _PAGES

_CACHE = {}


def _register_custom_ops():
    """Runtime-register the two custom DVE ops (idempotent)."""
    import concourse.dve_ops as dvo
    from concourse.dve_spec import (
        Spec, Src0, C0, C1, C2, C3, AluOp, scan, select, lower,
        Zero, One, _spill_c3_to_src1, _has_src1,
    )
    from concourse.dve_uop import DveOpSpec

    def reg(name, spec):
        for op in dvo.OPS:
            if op.name == name:
                return op
        row = dvo._CUSTOM_DVE_ROW_BASE + len(dvo.OPS)
        assert row < 0x20
        rd1 = _has_src1(spec)
        shas = {}
        for ver in ("v3", "v4"):
            u = lower(spec, ver=ver)
            shas[ver] = DveOpSpec(name=name, opcode=row, uops=u,
                                  rd1_en=rd1).sha(ver)
        op = dvo.DveOp(name, spec, subdim=False, uops_sha=shas)
        dvo.OPS.append(op)
        dvo._SUB_OPCODE_FOR_NAME[name] = row
        dvo.CUSTOM_DVE_SPECS[name] = spec
        return op

    # SCAN3: prefix-sum of 3-boundary pack (factored form fits 8 ALU stages)
    body3 = scan(
        AluOp.ADD,
        (Src0 > C0) + ((Src0 > C1) + (Src0 > C3) * C2) * C2,
    )
    spec3 = Spec(
        body=_spill_c3_to_src1(body3),
        reference=lambda in0, in1, s0, s1, imm2: np.cumsum(
            (in0 > s0).astype(np.float32)
            + (in0 > s1).astype(np.float32) * imm2
            + (in0 > in1).astype(np.float32) * imm2 * imm2,
            axis=-1, dtype=np.float32,
        ),
    )

    # MINMAX_SCAN: odd columns = running max, even columns = running min.
    # Col D-1 (odd) = max over all D elements; col D-2 (even) = min over
    # the first D-1 -- the caller folds in the last element separately.
    alt = scan(AluOp.MULTIPLY, Zero - One, init=One)   # (-1)^(k+1)
    rmax = scan(AluOp.MAX, Src0)
    rmin = scan(AluOp.MIN, Src0, init=C0)              # s0 = +3.4e38
    bodym = select(alt > Zero, rmax, rmin)

    def _ref_mm(in0, s0, s1, imm2):
        rmx = np.maximum.accumulate(in0, -1)
        rmn = np.minimum.accumulate(np.minimum(in0, s0), -1)
        k = np.arange(in0.shape[-1])
        return np.where(k % 2 == 1, rmx, rmn).astype(np.float32)

    specm = Spec(body=bodym, reference=_ref_mm)

    return reg("SCAN3_GT_ANT", spec3), reg("MINMAX_SCAN_ANT", specm)


def _build_program():
    import concourse.bacc as bacc
    import concourse.mybir as mybir
    import concourse.tile as tile

    op3, opm = _register_custom_ops()

    f32 = mybir.dt.float32
    bf16 = mybir.dt.bfloat16
    Alu = mybir.AluOpType
    Act = mybir.ActivationFunctionType

    nc = bacc.Bacc(None, target_bir_lowering=False)
    x = nc.dram_tensor("x", [_ROWS_PER_CORE, _D], f32, kind="ExternalInput")
    out = nc.dram_tensor("partial", [_P, 1], f32, kind="ExternalOutput")

    T = _TILES
    # fracs exactly as the reference: f32(k)/f32(10)
    fr = [float(np.float32(k) / np.float32(10.0)) for k in range(1, 10)]

    with tile.TileContext(nc) as tc:
        with tc.tile_pool(name="singles", bufs=1) as singles, \
             tc.tile_pool(name="xp", bufs=4) as xpool, \
             tc.tile_pool(name="mo", bufs=2) as mopool, \
             tc.tile_pool(name="so", bufs=4) as sopool, \
             tc.tile_pool(name="ascr", bufs=2) as ascr, \
             tc.tile_pool(name="small", bufs=4) as small:

            # persistent state
            fracs = singles.tile([_P, 9], f32)
            H = singles.tile([_P, 32 * T], f32)       # scan3 prefix harvests
            bposA = singles.tile([_P, 9 * T], f32)    # boundaries per tile
            deltaA = singles.tile([_P, T], f32)       # mx-mn per tile
            sgnacc = singles.tile([_P, 3 * T], f32)   # ACT sign accums
            c_all = singles.tile([_P, T * 11], f32)   # c_0..c_10 per tile
            ebias = singles.tile([_P, 1], f32)        # -e
            c3v = c_all[:].rearrange("p (t k) -> p t k", k=11)
            nc.gpsimd.memset(c3v[:, :, 0:1], float(_D))   # c_0 = 2048
            nc.gpsimd.memset(c3v[:, :, 10:11], 0.0)       # c_10 = 0
            for i, f in enumerate(fr):
                nc.gpsimd.memset(fracs[:, i:i + 1], f)
            nc.gpsimd.memset(ebias[:], -float(_E_F32))
            halfd = singles.tile([_P, 1], f32)        # D/2 for sign->count
            nc.gpsimd.memset(halfd[:], float(_D // 2))

            def counts_for(t, xt):
                def b(k):
                    return bposA[:, 9 * t + k - 1:9 * t + k]
                # DVE scan3 pass 1: (low=b1, mid=b7, high=b8)
                o1 = sopool.tile([_P, _D], f32, tag="so")
                nc.vector._custom_dve(
                    op3, out=o1[:], in0=xt[:, 0:_D],
                    s0=b(1), s1=b(7), in1=b(8), imm2=_M)
                o1v = o1[:].rearrange("p (s n) -> p s n", n=_PGSZ)
                nc.gpsimd.dma_start(
                    out=H[:, 32 * t:32 * t + 16], in_=o1v[:, :, _PGSZ - 1])
                # DVE scan3 pass 2: (low=b2, mid=b4, high=b9)
                o2 = sopool.tile([_P, _D], f32, tag="so")
                nc.vector._custom_dve(
                    op3, out=o2[:], in0=xt[:, 0:_D],
                    s0=b(2), s1=b(4), in1=b(9), imm2=_M)
                o2v = o2[:].rearrange("p (s n) -> p s n", n=_PGSZ)
                nc.gpsimd.dma_start(
                    out=H[:, 32 * t + 16:32 * t + 32], in_=o2v[:, :, _PGSZ - 1])
                # ACT sign passes: b3, b5, b6
                for i, k in enumerate((3, 5, 6)):
                    s = ascr.tile([_P, _D], bf16, tag="actscr")
                    nc.scalar.activation(
                        s[:], xt[:, 0:_D], Act.Sign,
                        bias=b(k), scale=-1.0,
                        accum_out=sgnacc[:, 3 * t + i:3 * t + i + 1])

            prev = None
            mnA = singles.tile([_P, T], f32)   # fixed-up per-tile min
            for t in range(T):
                xt = xpool.tile([_P, _D], f32, tag="xt")
                nc.sync.dma_start(out=xt[:], in_=x[t * _P:(t + 1) * _P, :])

                # fused min+max in one pass; col D-1 (odd) = full max,
                # col D-2 (even) = min of the first D-1 elements
                mo = mopool.tile([_P, _D], f32, tag="mo")
                nc.vector._custom_dve(
                    opm, out=mo[:], in0=xt[:], s0=3.0e38)
                mx = mo[:, _D - 1:_D]
                mn = mnA[:, t:t + 1]
                # fold the last element into the min
                nc.vector.tensor_scalar(mn, xt[:, _D - 1:_D],
                                        mo[:, _D - 2:_D - 1], None, Alu.min)

                # boundaries on DVE (same-engine dep with minmax => no
                # cross-engine roundtrip): delta = mx - mn; b = fracs*delta+mn
                delta = deltaA[:, t:t + 1]
                bpos = bposA[:, 9 * t:9 * t + 9]
                nc.vector.tensor_scalar(delta, mx, mn, None, Alu.subtract)
                nc.vector.tensor_scalar(bpos, fracs[:], delta, mn,
                                        Alu.mult, Alu.add)

                if prev is not None:
                    counts_for(t - 1, prev)
                prev = xt
            counts_for(T - 1, prev)

            # ---- epilogue ----
            # page diffs, biased by -OFS so every field-decode fraction is
            # within +-0.251 (a low field of exactly 128 = M/2 would
            # otherwise land on a round-half-to-even tie and corrupt the
            # decode): pd[:, g, s] = H[:, g, s] - H[:, g, s-1] - OFS
            OFS = 0.25 * _M * _M + 0.25 * _M  # 16448
            Hv = H[:].rearrange("p (g s) -> p g s", s=_PAGES)
            pd = singles.tile([_P, 32 * T], f32)
            pdv = pd[:].rearrange("p (g s) -> p g s", s=_PAGES)
            nc.vector.scalar_tensor_tensor(
                out=pdv[:, :, 1:_PAGES], in0=Hv[:, :, 1:_PAGES],
                scalar=float(OFS), in1=Hv[:, :, 0:_PAGES - 1],
                op0=Alu.subtract, op1=Alu.subtract)
            nc.vector.tensor_scalar(pdv[:, :, 0:1], Hv[:, :, 0:1],
                                    -float(OFS), None, Alu.add)
            # field decode with magic rounding
            # pd = low + M*mid + M^2*hi - OFS
            hi = singles.tile([_P, 32 * T], f32)
            mid = singles.tile([_P, 32 * T], f32)
            low = singles.tile([_P, 32 * T], f32)
            nc.vector.tensor_scalar(hi[:], pd[:], float(_M ** -2), _MAGIC,
                                    Alu.mult, Alu.add)
            nc.vector.tensor_scalar(hi[:], hi[:], -_MAGIC, None, Alu.add)
            nc.vector.scalar_tensor_tensor(
                out=pd[:], in0=hi[:], scalar=-float(_M * _M), in1=pd[:],
                op0=Alu.mult, op1=Alu.add)   # pd now = low + M*mid - OFS
            # mid: rnd((pd + OFS - M/4)/M) = rnd(pd/M + 64) - 64 + ...
            # (pd + 0.25*M^2)/M = mid + (low - M/4)/M, frac in [-0.25, 0.25]
            nc.vector.tensor_scalar(mid[:], pd[:], float(_M ** -1),
                                    _MAGIC + 0.25 * _M, Alu.mult, Alu.add)
            nc.vector.tensor_scalar(mid[:], mid[:], -_MAGIC, None, Alu.add)
            nc.vector.scalar_tensor_tensor(
                out=low[:], in0=mid[:], scalar=-float(_M), in1=pd[:],
                op0=Alu.mult, op1=Alu.add)   # low_true - OFS
            # page sums -> per-tile counts [128, (t,g)]
            lowsum = singles.tile([_P, 2 * T], f32)
            midsum = singles.tile([_P, 2 * T], f32)
            hisum = singles.tile([_P, 2 * T], f32)
            for src, dst in ((low, lowsum), (mid, midsum), (hi, hisum)):
                nc.vector.tensor_reduce(
                    out=dst[:],
                    in_=src[:].rearrange("p (g s) -> p g s", s=_PAGES),
                    axis=mybir.AxisListType.X, op=Alu.add)
            # ACT sign sums -> counts: c = 1024 - 0.5*S
            conv = singles.tile([_P, 3 * T], f32)
            nc.scalar.activation(conv[:], sgnacc[:], Act.Identity,
                                 bias=halfd[:], scale=-0.5)
            # assemble c_1..c_9 per tile
            ls = lowsum[:].rearrange("p (t g) -> p t g", g=2)
            ms = midsum[:].rearrange("p (t g) -> p t g", g=2)
            hs = hisum[:].rearrange("p (t g) -> p t g", g=2)
            cv = conv[:].rearrange("p (t i) -> p t i", i=3)
            # low sums carry -PAGES*OFS from the decode bias; re-add it here
            nc.vector.tensor_scalar(c3v[:, :, 1:2], ls[:, :, 0:1],
                                    float(_PAGES * OFS), None, Alu.add)  # c1
            nc.vector.tensor_scalar(c3v[:, :, 2:3], ls[:, :, 1:2],
                                    float(_PAGES * OFS), None, Alu.add)  # c2
            nc.vector.tensor_copy(c3v[:, :, 7:8], ms[:, :, 0:1])  # c7
            nc.vector.tensor_copy(c3v[:, :, 4:5], ms[:, :, 1:2])  # c4
            nc.vector.tensor_copy(c3v[:, :, 8:9], hs[:, :, 0:1])  # c8
            nc.vector.tensor_copy(c3v[:, :, 9:10], hs[:, :, 1:2])  # c9
            nc.vector.tensor_copy(c3v[:, :, 3:4], cv[:, :, 0:1])  # c3
            nc.vector.tensor_copy(c3v[:, :, 5:6], cv[:, :, 1:2])  # c5
            nc.vector.tensor_copy(c3v[:, :, 6:7], cv[:, :, 2:3])  # c6
            # obs_j = c_j - c_{j+1}
            obs = singles.tile([_P, T * 10], f32)
            obs3 = obs[:].rearrange("p (t j) -> p t j", j=10)
            nc.vector.tensor_tensor(out=obs3[:, :, 0:10], in0=c3v[:, :, 0:10],
                                    in1=c3v[:, :, 1:11], op=Alu.subtract)
            sq = singles.tile([_P, T * 10], f32)
            part = singles.tile([_P, 1], f32)
            nc.scalar.activation(sq[:], obs[:], Act.Square,
                                 bias=ebias[:], scale=1.0,
                                 accum_out=part[:])
            nc.sync.dma_start(out=out[:], in_=part[:])

    nc.compile()
    return nc


def _get_program():
    if "nc" not in _CACHE:
        _CACHE["nc"] = _build_program()
    return _CACHE["nc"]


def kernel(embeddings: np.ndarray) -> np.ndarray:
    from concourse.bass_utils import run_bass_kernel_spmd

    assert embeddings.shape == (_B_FULL, _D), embeddings.shape
    x = np.ascontiguousarray(embeddings, dtype=np.float32)
    nc = _get_program()
    in_maps = [
        {"x": x[c * _ROWS_PER_CORE:(c + 1) * _ROWS_PER_CORE]}
        for c in range(_N_CORES)
    ]
    res = run_bass_kernel_spmd(nc, in_maps, core_ids=list(range(_N_CORES)))
    total = np.float64(0.0)
    for r in res.results:
        total += r["partial"].astype(np.float64).sum()
    mean_chi2 = total / np.float64(_E_F32) / np.float64(_B_FULL)
    return np.float32(mean_chi2)


# revision 19
# speedup vs baseline: 1.0491x; 1.0491x over previous
"""ChiSquareLoss kernel for Trainium2 (8 NeuronCores, SPMD).

Problem (see reference): for each of B=16384 rows of a [B, 2048] f32 matrix,
build a 10-bin histogram between the row's min and max, then
chi2_row = sum_j (obs_j - e)^2 / (e + eps) with e = B/10, and return
mean(chi2_row).

Per row: bin index of x is #{k in 1..9 : x > b_k}, b_k = mn + (mx-mn)*k/10,
so the histogram follows from the cumulative counts c_k = #{x > b_k}:
obs_j = c_j - c_{j+1}, c_0 = 2048, c_10 = 0.

Engine strategy (each core: 16 tiles of [128 rows, 2048]):
  All DVE reduction-class ops run at 1x (~2.3-2.4us per [128,2048] pass) on
  TRN2 regardless of dtype (accumulate/scan uops exist only for 1x), ACT at
  (N+352)/1.2ns + 280ns accumulator read, so the kernel minimizes the NUMBER
  of full passes over the data using two runtime-registered custom DVE ops:

  MINMAX_SCAN (1 pass = row min AND max): out[k] = parity-select between
    running-min and running-max scans; the tile carries one extra sentinel
    column (-3e38, neutral for max) so col 2047 (odd) holds min of all 2048
    elements and col 2048 (even) holds max. One strided 2-column harvest
    copy recovers [mn, mx].

  SCAN3 (1 pass = 3 counts): out[k] = prefix-sum of
    (x>b_lo) + 256*((x>b_mid) + 256*(x>b_hi)); harvesting the prefix at the
    16 page boundaries (pages of 128) and differencing gives per-page packed
    counts, exact in fp32 because per-page fields (<=128) stay below 256 and
    the row-level prefix stays < 2^24 when the mid/high fields are assigned
    to the small-count (upper) boundaries. Groups: (b1,b7,b8) and (b2,b4,b9)
    on DVE; b3, b5, b6 counted on ACT via the Sign-activation accumulator
    (c = 1024 - S/2).

  Per tile: DVE = minmax_scan + 2 scan3 + 3 tiny harvests (~7.8us),
  ACT = boundary affine ops + 3 sign passes (~7.2us); a one-tile software
  pipeline keeps both engines busy. Epilogue: bulk page-diff + field decode
  (magic-constant rounding) + per-tile page sums -> c_k, obs diffs, one ACT
  Square pass with accumulator -> per-partition partial sums.
  Host: total / (e + eps) / B.
"""

import numpy as np

_B_FULL = 16384
_D = 2048
_N_CORES = 8
_ROWS_PER_CORE = _B_FULL // _N_CORES  # 2048
_P = 128
_TILES = _ROWS_PER_CORE // _P  # 16
_BINS = 10
_E_F32 = np.float32(_B_FULL / _BINS)  # 1638.4f

_MAGIC = float(np.float32(2 ** 23 + 2 ** 22))  # round-to-int magic for fp32
_M = 256.0        # scan3 packing multiplier
_PAGES = 16       # pages of 128 elements
_PGSZ = _D // _PAGES

_CACHE = {}


def _register_custom_ops():
    """Runtime-register the two custom DVE ops (idempotent)."""
    import concourse.dve_ops as dvo
    from concourse.dve_spec import (
        Spec, Src0, C0, C1, C2, C3, AluOp, scan, select, lower,
        Zero, One, _spill_c3_to_src1, _has_src1,
    )
    from concourse.dve_uop import DveOpSpec

    def reg(name, spec):
        for op in dvo.OPS:
            if op.name == name:
                return op
        row = dvo._CUSTOM_DVE_ROW_BASE + len(dvo.OPS)
        assert row < 0x20
        rd1 = _has_src1(spec)
        shas = {}
        for ver in ("v3", "v4"):
            u = lower(spec, ver=ver)
            shas[ver] = DveOpSpec(name=name, opcode=row, uops=u,
                                  rd1_en=rd1).sha(ver)
        op = dvo.DveOp(name, spec, subdim=False, uops_sha=shas)
        dvo.OPS.append(op)
        dvo._SUB_OPCODE_FOR_NAME[name] = row
        dvo.CUSTOM_DVE_SPECS[name] = spec
        return op

    # SCAN3: prefix-sum of 3-boundary pack (factored form fits 8 ALU stages)
    body3 = scan(
        AluOp.ADD,
        (Src0 > C0) + ((Src0 > C1) + (Src0 > C3) * C2) * C2,
    )
    spec3 = Spec(
        body=_spill_c3_to_src1(body3),
        reference=lambda in0, in1, s0, s1, imm2: np.cumsum(
            (in0 > s0).astype(np.float32)
            + (in0 > s1).astype(np.float32) * imm2
            + (in0 > in1).astype(np.float32) * imm2 * imm2,
            axis=-1, dtype=np.float32,
        ),
    )

    # MINMAX_SCAN: odd columns = running min, even columns = running max.
    # The caller appends one sentinel column (-3e38, neutral for max) so
    # col D-1 (odd) = min over all D elements, col D (even) = max.
    alt = scan(AluOp.MULTIPLY, Zero - One, init=One)   # (-1)^(k+1)
    rmax = scan(AluOp.MAX, Src0)
    rmin = scan(AluOp.MIN, Src0, init=C0)              # s0 = +3.4e38
    bodym = select(alt > Zero, rmin, rmax)

    def _ref_mm(in0, s0, s1, imm2):
        rmx = np.maximum.accumulate(in0, -1)
        rmn = np.minimum.accumulate(np.minimum(in0, s0), -1)
        k = np.arange(in0.shape[-1])
        return np.where(k % 2 == 1, rmn, rmx).astype(np.float32)

    specm = Spec(body=bodym, reference=_ref_mm)

    return reg("SCAN3_GT_ANT", spec3), reg("MINMAX_SCAN_ANT", specm)


def _build_program():
    import concourse.bacc as bacc
    import concourse.mybir as mybir
    import concourse.tile as tile

    op3, opm = _register_custom_ops()

    f32 = mybir.dt.float32
    bf16 = mybir.dt.bfloat16
    Alu = mybir.AluOpType
    Act = mybir.ActivationFunctionType

    nc = bacc.Bacc(None, target_bir_lowering=False)
    x = nc.dram_tensor("x", [_ROWS_PER_CORE, _D], f32, kind="ExternalInput")
    out = nc.dram_tensor("partial", [_P, 1], f32, kind="ExternalOutput")

    T = _TILES
    # fracs exactly as the reference: f32(k)/f32(10)
    fr = [float(np.float32(k) / np.float32(10.0)) for k in range(1, 10)]

    with tile.TileContext(nc) as tc:
        with tc.tile_pool(name="singles", bufs=1) as singles, \
             tc.tile_pool(name="xp", bufs=4) as xpool, \
             tc.tile_pool(name="mo", bufs=2) as mopool, \
             tc.tile_pool(name="so", bufs=4) as sopool, \
             tc.tile_pool(name="ascr", bufs=2) as ascr, \
             tc.tile_pool(name="small", bufs=4) as small:

            # persistent state
            fracs = singles.tile([_P, 9], f32)
            H = singles.tile([_P, 32 * T], f32)       # scan3 prefix harvests
            bposA = singles.tile([_P, 9 * T], f32)    # boundaries per tile
            deltaA = singles.tile([_P, T], f32)       # mx-mn per tile
            sgnacc = singles.tile([_P, 3 * T], f32)   # ACT sign accums
            c_all = singles.tile([_P, T * 11], f32)   # c_0..c_10 per tile
            ebias = singles.tile([_P, 1], f32)        # -e
            c3v = c_all[:].rearrange("p (t k) -> p t k", k=11)
            nc.gpsimd.memset(c3v[:, :, 0:1], float(_D))   # c_0 = 2048
            nc.gpsimd.memset(c3v[:, :, 10:11], 0.0)       # c_10 = 0
            for i, f in enumerate(fr):
                nc.gpsimd.memset(fracs[:, i:i + 1], f)
            nc.gpsimd.memset(ebias[:], -float(_E_F32))
            halfd = singles.tile([_P, 1], f32)        # D/2 for sign->count
            nc.gpsimd.memset(halfd[:], float(_D // 2))

            def counts_for(t, xt):
                def b(k):
                    return bposA[:, 9 * t + k - 1:9 * t + k]
                # DVE scan3 pass 1: (low=b1, mid=b7, high=b8)
                o1 = sopool.tile([_P, _D], f32, tag="so")
                nc.vector._custom_dve(
                    op3, out=o1[:], in0=xt[:, 0:_D],
                    s0=b(1), s1=b(7), in1=b(8), imm2=_M)
                o1v = o1[:].rearrange("p (s n) -> p s n", n=_PGSZ)
                nc.vector.tensor_copy(
                    H[:, 32 * t:32 * t + 16], o1v[:, :, _PGSZ - 1])
                # DVE scan3 pass 2: (low=b2, mid=b4, high=b9)
                o2 = sopool.tile([_P, _D], f32, tag="so")
                nc.vector._custom_dve(
                    op3, out=o2[:], in0=xt[:, 0:_D],
                    s0=b(2), s1=b(4), in1=b(9), imm2=_M)
                o2v = o2[:].rearrange("p (s n) -> p s n", n=_PGSZ)
                nc.vector.tensor_copy(
                    H[:, 32 * t + 16:32 * t + 32], o2v[:, :, _PGSZ - 1])
                # ACT sign passes: b3, b5, b6
                for i, k in enumerate((3, 5, 6)):
                    s = ascr.tile([_P, _D], bf16, tag="actscr")
                    nc.scalar.activation(
                        s[:], xt[:, 0:_D], Act.Sign,
                        bias=b(k), scale=-1.0,
                        accum_out=sgnacc[:, 3 * t + i:3 * t + i + 1])

            mm = singles.tile([_P, 2 * T], f32)       # [mn, mx] per tile
            prev = None
            for t in range(T):
                xt = xpool.tile([_P, _D + 2], f32, tag="xt")
                nc.gpsimd.memset(xt[:, _D:_D + 1], -3.0e38)  # max-neutral
                nc.sync.dma_start(out=xt[:, 0:_D], in_=x[t * _P:(t + 1) * _P, :])

                # fused min+max in one pass; harvest [mn, mx] in one copy
                mo = mopool.tile([_P, _D + 2], f32, tag="mo")
                nc.vector._custom_dve(
                    opm, out=mo[:, 0:_D + 1], in0=xt[:, 0:_D + 1], s0=3.0e38)
                nc.vector.tensor_copy(
                    mm[:, 2 * t:2 * t + 2], mo[:, _D - 1:_D + 1])

                # boundaries on ACT: delta = mx - mn; b_k = frac_k*delta + mn
                mn = mm[:, 2 * t:2 * t + 1]
                mx = mm[:, 2 * t + 1:2 * t + 2]
                delta = deltaA[:, t:t + 1]
                bpos = bposA[:, 9 * t:9 * t + 9]
                nc.scalar.activation(delta, mn, Act.Identity,
                                     bias=mx, scale=-1.0)
                nc.scalar.activation(bpos, fracs[:], Act.Identity,
                                     bias=mn, scale=delta)

                if prev is not None:
                    counts_for(t - 1, prev)
                prev = xt
            counts_for(T - 1, prev)

            # ---- epilogue ----
            # page diffs, biased by -OFS so every field-decode fraction is
            # within +-0.251 (a low field of exactly 128 = M/2 would
            # otherwise land on a round-half-to-even tie and corrupt the
            # decode): pd[:, g, s] = H[:, g, s] - H[:, g, s-1] - OFS
            OFS = 0.25 * _M * _M + 0.25 * _M  # 16448
            Hv = H[:].rearrange("p (g s) -> p g s", s=_PAGES)
            pd = singles.tile([_P, 32 * T], f32)
            pdv = pd[:].rearrange("p (g s) -> p g s", s=_PAGES)
            nc.vector.scalar_tensor_tensor(
                out=pdv[:, :, 1:_PAGES], in0=Hv[:, :, 1:_PAGES],
                scalar=float(OFS), in1=Hv[:, :, 0:_PAGES - 1],
                op0=Alu.subtract, op1=Alu.subtract)
            nc.vector.tensor_scalar(pdv[:, :, 0:1], Hv[:, :, 0:1],
                                    -float(OFS), None, Alu.add)
            # field decode with magic rounding
            # pd = low + M*mid + M^2*hi - OFS
            hi = singles.tile([_P, 32 * T], f32)
            mid = singles.tile([_P, 32 * T], f32)
            low = singles.tile([_P, 32 * T], f32)
            nc.vector.tensor_scalar(hi[:], pd[:], float(_M ** -2), _MAGIC,
                                    Alu.mult, Alu.add)
            nc.vector.tensor_scalar(hi[:], hi[:], -_MAGIC, None, Alu.add)
            nc.vector.scalar_tensor_tensor(
                out=pd[:], in0=hi[:], scalar=-float(_M * _M), in1=pd[:],
                op0=Alu.mult, op1=Alu.add)   # pd now = low + M*mid - OFS
            # mid: rnd((pd + OFS - M/4)/M) = rnd(pd/M + 64) - 64 + ...
            # (pd + 0.25*M^2)/M = mid + (low - M/4)/M, frac in [-0.25, 0.25]
            nc.vector.tensor_scalar(mid[:], pd[:], float(_M ** -1),
                                    _MAGIC + 0.25 * _M, Alu.mult, Alu.add)
            nc.vector.tensor_scalar(mid[:], mid[:], -_MAGIC, None, Alu.add)
            nc.vector.scalar_tensor_tensor(
                out=low[:], in0=mid[:], scalar=-float(_M), in1=pd[:],
                op0=Alu.mult, op1=Alu.add)   # low_true - OFS
            # page sums -> per-tile counts [128, (t,g)]
            lowsum = singles.tile([_P, 2 * T], f32)
            midsum = singles.tile([_P, 2 * T], f32)
            hisum = singles.tile([_P, 2 * T], f32)
            for src, dst in ((low, lowsum), (mid, midsum), (hi, hisum)):
                nc.vector.tensor_reduce(
                    out=dst[:],
                    in_=src[:].rearrange("p (g s) -> p g s", s=_PAGES),
                    axis=mybir.AxisListType.X, op=Alu.add)
            # ACT sign sums -> counts: c = 1024 - 0.5*S
            conv = singles.tile([_P, 3 * T], f32)
            nc.scalar.activation(conv[:], sgnacc[:], Act.Identity,
                                 bias=halfd[:], scale=-0.5)
            # assemble c_1..c_9 per tile
            ls = lowsum[:].rearrange("p (t g) -> p t g", g=2)
            ms = midsum[:].rearrange("p (t g) -> p t g", g=2)
            hs = hisum[:].rearrange("p (t g) -> p t g", g=2)
            cv = conv[:].rearrange("p (t i) -> p t i", i=3)
            # low sums carry -PAGES*OFS from the decode bias; re-add it here
            nc.vector.tensor_scalar(c3v[:, :, 1:2], ls[:, :, 0:1],
                                    float(_PAGES * OFS), None, Alu.add)  # c1
            nc.vector.tensor_scalar(c3v[:, :, 2:3], ls[:, :, 1:2],
                                    float(_PAGES * OFS), None, Alu.add)  # c2
            nc.vector.tensor_copy(c3v[:, :, 7:8], ms[:, :, 0:1])  # c7
            nc.vector.tensor_copy(c3v[:, :, 4:5], ms[:, :, 1:2])  # c4
            nc.vector.tensor_copy(c3v[:, :, 8:9], hs[:, :, 0:1])  # c8
            nc.vector.tensor_copy(c3v[:, :, 9:10], hs[:, :, 1:2])  # c9
            nc.vector.tensor_copy(c3v[:, :, 3:4], cv[:, :, 0:1])  # c3
            nc.vector.tensor_copy(c3v[:, :, 5:6], cv[:, :, 1:2])  # c5
            nc.vector.tensor_copy(c3v[:, :, 6:7], cv[:, :, 2:3])  # c6
            # obs_j = c_j - c_{j+1}
            obs = singles.tile([_P, T * 10], f32)
            obs3 = obs[:].rearrange("p (t j) -> p t j", j=10)
            nc.vector.tensor_tensor(out=obs3[:, :, 0:10], in0=c3v[:, :, 0:10],
                                    in1=c3v[:, :, 1:11], op=Alu.subtract)
            sq = singles.tile([_P, T * 10], f32)
            part = singles.tile([_P, 1], f32)
            nc.scalar.activation(sq[:], obs[:], Act.Square,
                                 bias=ebias[:], scale=1.0,
                                 accum_out=part[:])
            nc.sync.dma_start(out=out[:], in_=part[:])

    nc.compile()
    return nc


def _get_program():
    if "nc" not in _CACHE:
        _CACHE["nc"] = _build_program()
    return _CACHE["nc"]


def kernel(embeddings: np.ndarray) -> np.ndarray:
    from concourse.bass_utils import run_bass_kernel_spmd

    assert embeddings.shape == (_B_FULL, _D), embeddings.shape
    x = np.ascontiguousarray(embeddings, dtype=np.float32)
    nc = _get_program()
    in_maps = [
        {"x": x[c * _ROWS_PER_CORE:(c + 1) * _ROWS_PER_CORE]}
        for c in range(_N_CORES)
    ]
    res = run_bass_kernel_spmd(nc, in_maps, core_ids=list(range(_N_CORES)))
    total = np.float64(0.0)
    for r in res.results:
        total += r["partial"].astype(np.float64).sum()
    mean_chi2 = total / np.float64(_E_F32) / np.float64(_B_FULL)
    return np.float32(mean_chi2)
